# revision 11
# baseline (speedup 1.0000x reference)
"""Trainium2 Bass kernel for causal self-attention with RoPE (mixed variant).

Sharding: tensor-parallel over heads x data-parallel over batch.
8 cores = 2 batches x 4 head-groups (4 heads each). Each core computes
qkv for its heads from x[b], RoPE, causal attention, and a partial
projection y_part = attn_out_g @ w_proj[rows_g]. Host sums the 4
partials per batch (bf16 partials, f32 accumulate on host).

Per-core device pipeline (all matmuls bf16 with f32 PSUM accumulate):
  A) qk^T = W_qk^T @ x^T   -> [d, t] layout; RoPE applied in [d, t] via
     pair-swapped copy (even/odd partition swap) + cos/sin tables.
  B) v = x @ W_v           -> [t, d] layout (x^T-stationary matmuls),
     with a ones-column appended per head (denominator trick).
  C) per head: S^T tiles = k^T.T @ q^T (K=64), exp on ScalarE
     (scale=1/8 fused), causal diag-tile mask via DVE multiply with an
     upper-tri 0/1 tile, P^T @ [V|1] accumulates O'^T = [O^T; denom]
     in PSUM. Normalize by 1/denom (broadcast via gpsimd) -> O^T bf16.
  D) y_part = O^T.T @ W_p rows, bf16 out; first half interleaved into
     phase C's second query-window pass so its matmuls+DMA overlap the
     ACT-bound attention stretch.
"""

import numpy as np
import ml_dtypes
from contextlib import ExitStack

B, T, C = 2, 2048, 1024
NH, HD = 16, 64
NCORES = 8
GROUPS = 4            # head-groups (tensor parallel axis)
HPG = NH // GROUPS    # heads per group = 4
DG = HPG * HD         # 256 cols per group for q (and k, v)
CT = C // 128         # 8 contraction tiles
NTT = T // 128        # 16 t-tiles
MASK_NEG = -30000.0

bf16 = ml_dtypes.bfloat16

_CACHE: dict = {}


def _emit(tc, nc, mybir, bass, ctx):
    dt = mybir.dt
    f32, b16 = dt.float32, dt.bfloat16
    AF = mybir.ActivationFunctionType

    xT_d = nc.dram_tensor("xT", [C, T], b16, kind="ExternalInput")
    wqk_d = nc.dram_tensor("wqk", [C, 2 * DG], b16, kind="ExternalInput")
    wv_d = nc.dram_tensor("wv", [C, DG], b16, kind="ExternalInput")
    wp_d = nc.dram_tensor("wp", [DG, C], b16, kind="ExternalInput")
    cos_d = nc.dram_tensor("cosT", [128, T], b16, kind="ExternalInput")
    sin_d = nc.dram_tensor("sinT", [128, T], b16, kind="ExternalInput")
    tri_d = nc.dram_tensor("tri", [128, 128], b16, kind="ExternalInput")
    y_d = nc.dram_tensor("y", [T, C], b16, kind="ExternalOutput")

    const = ctx.enter_context(tc.tile_pool(name="const", bufs=1))
    work = ctx.enter_context(tc.tile_pool(name="work", bufs=1))

    # ---- resident SBUF loads, ordered by first use ----
    xt_sb = const.tile([128, CT, T], b16, tag="xt")
    wqk_sb = const.tile([128, CT, 2 * DG], b16, tag="wqk")
    wv_sb = const.tile([128, CT, DG], b16, tag="wv")
    TH = T // 2
    for i in range(CT):
        # wqk + first-half xT land first so phase A half-0 starts early
        nc.sync.dma_start(wqk_sb[:, i, :], wqk_d.ap()[i * 128:(i + 1) * 128, :])
        nc.sync.dma_start(xt_sb[:, i, 0:TH],
                          xT_d.ap()[i * 128:(i + 1) * 128, 0:TH])
    cos_sb = const.tile([128, T], b16, tag="cos")
    nc.sync.dma_start(cos_sb[:], cos_d.ap())
    sin_sb = const.tile([128, T], b16, tag="sin")
    nc.sync.dma_start(sin_sb[:], sin_d.ap())
    for i in range(CT):
        nc.sync.dma_start(xt_sb[:, i, TH:T],
                          xT_d.ap()[i * 128:(i + 1) * 128, TH:T])
        nc.sync.dma_start(wv_sb[:, i, :], wv_d.ap()[i * 128:(i + 1) * 128, :])
    tri_sb = const.tile([128, 128], b16, tag="tri")
    nc.sync.dma_start(tri_sb[:], tri_d.ap())
    wp_sb = const.tile([128, 2, C], b16, tag="wp")
    nc.sync.dma_start(wp_sb[:], wp_d.ap().rearrange("(a p) d -> p a d", p=128))

    # rope outputs: [d, t] bf16, 2 grp-tiles each (grp = 2 heads = 128 rows)
    q_sb = work.tile([128, 2, T], b16, tag="q")
    k_sb = work.tile([128, 2, T], b16, tag="k")
    # v in [t, d] layout with per-head ones column: [t-tile, head, 65]
    v_sb = work.tile([128, NTT, HPG, HD + 1], b16, tag="v")
    # attention outputs O^T (normalized), split by query half for clean
    # tile-level deps (phase D half 0 only needs o_lo)
    o_lo = work.tile([128, 2, TH], b16, tag="olo")
    o_hi = work.tile([128, 2, TH], b16, tag="ohi")

    # ones columns only (v cols 0:64 written by phase B evacuation)
    nc.gpsimd.memset(v_sb[:, :, :, 64:65], 1.0)

    # ---- phase A: qk^T matmuls + rope;  phase B: v matmuls ----
    with (
        tc.tile_pool(name="qk_ps", bufs=3, space="PSUM") as qk_pool,
        tc.tile_pool(name="v_ps", bufs=2, space="PSUM") as v_pool,
        tc.tile_pool(name="rope", bufs=2) as rope_pool,
    ):
        for half in range(2):      # [128, 1024] halves
            h0 = half * TH
            hsl = slice(h0, h0 + TH)
            for dpair in ((0, 2), (1, 3)):  # (q, k) per grp together
                # ci-outer over a dtile pair: each arriving c-tile feeds 4
                # matmuls, so the DMA-paced kernel start keeps PE fed
                pss = {}
                for d in dpair:
                    qkps = qk_pool.tile([128, TH], f32, tag="qkps")
                    pss[d] = qkps
                for ci in range(CT):
                    for d in dpair:
                        for j in range(2):
                            nc.tensor.matmul(
                                pss[d][:, j * 512:(j + 1) * 512],
                                wqk_sb[:, ci, d * 128:(d + 1) * 128],
                                xt_sb[:, ci, h0 + j * 512:h0 + (j + 1) * 512],
                                start=(ci == 0),
                                stop=(ci == CT - 1),
                            )
                for d in dpair:
                    is_q = d < 2
                    grp = d % 2
                    ps = pss[d]
                    # evacuate to bf16 SBUF (ScalarE, closer to PSUM)
                    raw = rope_pool.tile([128, TH], b16, tag="raw")
                    nc.scalar.copy(raw[:], ps[:])
                    # pair-swap partitions (d even<->odd): 32-way shuffle
                    shuf = rope_pool.tile([128, TH], b16, tag="shuf")
                    nc.vector.stream_shuffle(shuf[:], raw[:],
                                             [i ^ 1 for i in range(32)])
                    # rope: out = raw*cos + shuf*sin'  (t2 on gpsimd to
                    # keep DVE off the phase A->C critical path)
                    t1 = rope_pool.tile([128, TH], b16, tag="t1")
                    nc.vector.tensor_mul(t1[:], raw[:], cos_sb[:, hsl])
                    t2 = rope_pool.tile([128, TH], b16, tag="t2")
                    nc.gpsimd.tensor_mul(t2[:], shuf[:], sin_sb[:, hsl])
                    dst = (q_sb if is_q else k_sb)
                    nc.vector.tensor_add(dst[:, grp, hsl], t1[:], t2[:])

        # phase B: v in [t, d] layout (first half; rest interleaved into C)
        for tt in range(NTT // 2):
            vps = v_pool.tile([128, DG], f32, tag="vps")
            for ci in range(CT):
                nc.tensor.matmul(
                    vps[:],
                    xt_sb[:, ci, tt * 128:(tt + 1) * 128],
                    wv_sb[:, ci, :],
                    start=(ci == 0),
                    stop=(ci == CT - 1),
                )
            nc.scalar.copy(
                v_sb[:, tt, :, 0:HD],
                vps[:].rearrange("p (h d) -> p h d", h=HPG),
            )

    # ---- phase C: attention per head; phase D interleaved ----
    # PSUM budget: sps 2 bufs x 2 banks + ops 1 buf x 2 banks x... exact:
    # sps [128,1024] f32 = 2 banks (bufs=2 -> 4), ops [65,1024] f32 = 2
    # banks (bufs=2 -> 4)... that is 8; y interleave needs its own pool,
    # so ops gets bufs=1 (2 banks) and y_ps bufs=2 (2 banks).
    with (
        tc.tile_pool(name="o_ps", bufs=1, space="PSUM") as o_pool,
        tc.tile_pool(name="s_ps", bufs=2, space="PSUM") as s_pool,
        tc.tile_pool(name="y_ps", bufs=2, space="PSUM") as y_pool,
        tc.tile_pool(name="p_sb", bufs=6) as p_pool,
        tc.tile_pool(name="r_sb", bufs=2) as r_pool,
        tc.tile_pool(name="y_sb", bufs=4) as ysb_pool,
    ):
        def emit_proj(tt, cc, on_dve):
            o_t = o_lo if tt < NTT // 2 else o_hi
            toff = tt * 128 - (0 if tt < NTT // 2 else TH)
            yps = y_pool.tile([128, 512], f32, tag="yps")
            for grp in range(2):
                nc.tensor.matmul(
                    yps[:],
                    o_t[:, grp, toff:toff + 128],
                    wp_sb[:, grp, cc * 512:(cc + 1) * 512],
                    start=(grp == 0),
                    stop=(grp == 1),
                )
            ysb = ysb_pool.tile([128, 512], b16, tag="ysb")
            if on_dve:
                nc.vector.tensor_copy(ysb[:], yps[:])
            else:
                nc.scalar.copy(ysb[:], yps[:])
            nc.sync.dma_start(
                y_d.ap()[tt * 128:(tt + 1) * 128, cc * 512:(cc + 1) * 512],
                ysb[:],
            )

        for jh in range(2):  # 1024-wide q windows (2 x 512 sub-chunks)
            for h in range(HPG):
                grp, base = h // 2, 64 * (h % 2)
                o_t = o_lo if jh == 0 else o_hi
                ops = o_pool.tile([65, 1024], f32, tag="ops")
                w0 = jh * 1024
                ilim = min(8 * jh + 8, NTT)
                for i in range(ilim):
                    woff = max(0, 128 * i - w0)  # first valid col in window
                    sps = s_pool.tile([128, 1024], f32, tag="sps")
                    klhs = k_sb[base:base + 64, grp, i * 128:(i + 1) * 128]
                    for sj in range(2):  # 512 sub-chunks (PSUM bank each)
                        j = 2 * jh + sj
                        if i > 4 * j + 3:
                            continue  # fully masked sub-chunk
                        off = max(0, 128 * i - 512 * j)
                        nc.tensor.matmul(
                            sps[:, sj * 512 + off:(sj + 1) * 512],
                            klhs,
                            q_sb[base:base + 64, grp,
                                 j * 512 + off:(j + 1) * 512],
                            start=True,
                            stop=True,
                        )
                    psb = p_pool.tile([128, 1024], b16, tag="psb")
                    nc.scalar.activation(
                        psb[:, woff:1024], sps[:, woff:1024], AF.Exp,
                        scale=0.125,
                    )
                    d0 = 128 * i - w0  # tri-block col within window
                    if 0 <= d0 <= 1024 - 128:
                        # zero the strictly-lower (q < key) part of the
                        # diagonal tile post-exp (replaces mask matmul)
                        nc.vector.tensor_mul(psb[:, d0:d0 + 128],
                                             psb[:, d0:d0 + 128], tri_sb[:])
                    for sj in range(2):
                        j = 2 * jh + sj
                        if i > 4 * j + 3:
                            continue
                        off = max(0, 128 * i - 512 * j)
                        nc.tensor.matmul(
                            ops[:, sj * 512 + off:(sj + 1) * 512],
                            v_sb[:, i, h, :],
                            psb[:, sj * 512 + off:(sj + 1) * 512],
                            start=(i == 0),
                            stop=(i == min(4 * j + 3, ilim - 1)),
                        )
                # normalize this 1024-col window: O^T * (1/denom)
                wsl = slice(w0 - jh * 1024, w0 - jh * 1024 + 1024)
                rec = r_pool.tile([1, 1024], dt.float32, tag="rec")
                nc.vector.reciprocal(rec[:], ops[64:65, :])
                rrep = r_pool.tile([64, 1024], dt.float32, tag="rrep")
                nc.gpsimd.partition_broadcast(rrep[:], rec[:])
                nc.vector.tensor_mul(o_t[base:base + 64, grp, wsl],
                                     ops[0:64, :], rrep[:])
                if jh == 1:
                    # interleave phase-D half 0 (reads o_lo only) into the
                    # ACT-bound attention stretch
                    for tt in (2 * h, 2 * h + 1):
                        emit_proj(tt, 0, True)
                        emit_proj(tt, 1, True)

        # ---- phase D tail: second query half ----
        for tt in range(NTT // 2, NTT):
            for cc in range(2):
                emit_proj(tt, cc, cc == 1)


def build_program():
    if "nc" in _CACHE:
        return _CACHE["nc"]
    import concourse.bass as bass
    import concourse.bacc as bacc
    import concourse.tile as tile
    import concourse.mybir as mybir

    nc = bacc.Bacc("TRN2", target_bir_lowering=False, debug=False,
                   enable_asserts=True)
    with tile.TileContext(nc) as tc:
        with ExitStack() as ctx:
            _emit(tc, nc, mybir, bass, ctx)
    nc.compile()
    _CACHE["nc"] = nc
    return nc


def make_tables():
    """cos/sin tables ([128, T], two 64-row head copies) and tri mask."""
    if "tables" in _CACHE:
        return _CACHE["tables"]
    hd = HD
    inv_freq = 1.0 / (10000.0 ** (np.arange(0, hd, 2, dtype=np.float64) / hd))
    t = np.arange(T, dtype=np.float64)
    emb = t[:, None] * np.concatenate([inv_freq, inv_freq])[None, :]  # [T, 64]
    cos = np.cos(emb).T.astype(np.float32)       # [64, T]
    sin = np.sin(emb).T.astype(np.float32)
    sign = np.where(np.arange(hd) % 2 == 0, -1.0, 1.0).astype(np.float32)
    sin = sin * sign[:, None]
    cos128 = np.concatenate([cos, cos], axis=0).astype(bf16)   # [128, T]
    sin128 = np.concatenate([sin, sin], axis=0).astype(bf16)
    ii = np.arange(128)
    # tri[k, q] = 1 where q >= k (valid causal), else 0
    tri = (ii[None, :] >= ii[:, None]).astype(bf16)
    _CACHE["tables"] = (cos128, sin128, tri)
    return _CACHE["tables"]


def make_in_maps(x, w_qkv, w_proj):
    cos128, sin128, tri = make_tables()
    wq = w_qkv[:, 0:C]
    wk = w_qkv[:, C:2 * C]
    wv = w_qkv[:, 2 * C:3 * C]
    in_maps = []
    for b in range(B):
        xT = np.ascontiguousarray(x[b].T).astype(bf16)
        for g in range(GROUPS):
            sl = slice(g * DG, (g + 1) * DG)
            in_maps.append({
                "xT": xT,
                "wqk": np.concatenate([wq[:, sl], wk[:, sl]], axis=1).astype(bf16),
                "wv": wv[:, sl].astype(bf16),
                "wp": w_proj[sl, :].astype(bf16),
                "cosT": cos128, "sinT": sin128, "tri": tri,
            })
    return in_maps


def kernel(x, w_qkv, w_proj):
    from concourse import bass_utils
    nc = build_program()
    in_maps = make_in_maps(np.asarray(x, dtype=np.float32),
                           np.asarray(w_qkv, dtype=np.float32),
                           np.asarray(w_proj, dtype=np.float32))
    res = bass_utils.run_bass_kernel_spmd(nc, in_maps, list(range(NCORES)))
    out = np.empty((B, T, C), dtype=np.float32)
    for b in range(B):
        acc = np.zeros((T, C), dtype=np.float32)
        for g in range(GROUPS):
            acc += np.asarray(res.results[b * GROUPS + g]["y"], dtype=np.float32)
        out[b] = acc
    return out


# revision 30
# speedup vs baseline: 1.3979x; 1.3979x over previous
"""Trainium2 Bass kernel for causal self-attention with RoPE (mixed variant).

Sharding: tensor-parallel over heads x data-parallel over batch.
8 cores = 2 batches x 4 head-groups (4 heads each). Each core computes
qkv for its heads from x[b], RoPE, causal attention, and a partial
projection y_part = attn_out_g @ w_proj[rows_g]. Host sums the 4
partials per batch (bf16 partials, f32 accumulate on host).

Per-core device pipeline (all matmuls bf16 with f32 PSUM accumulate):
  A) qk^T = W_qk^T @ x^T   -> [d, t] layout; RoPE applied in [d, t] via
     pair-swapped copy (even/odd partition swap) + cos/sin tables.
  B) v = x @ W_v           -> [t, d] layout (x^T-stationary matmuls),
     with a ones-column appended per head (denominator trick).
  C) per head: S^T tiles = k^T.T @ q^T (K=64), exp on ScalarE
     (scale=1/8 fused), causal diag-tile mask via DVE multiply with an
     upper-tri 0/1 tile, P^T @ [V|1] accumulates O'^T = [O^T; denom]
     in PSUM. Normalize by 1/denom (broadcast via gpsimd) -> O^T bf16.
  D) y_part = O^T.T @ W_p rows, bf16 out; first half interleaved into
     phase C's second query-window pass so its matmuls+DMA overlap the
     ACT-bound attention stretch.
"""

import numpy as np
import ml_dtypes
from contextlib import ExitStack

B, T, C = 2, 2048, 1024
NH, HD = 16, 64
NCORES = 8
GROUPS = 4            # head-groups (tensor parallel axis)
HPG = NH // GROUPS    # heads per group = 4
DG = HPG * HD         # 256 cols per group for q (and k, v)
CT = C // 128         # 8 contraction tiles
NTT = T // 128        # 16 t-tiles
MASK_NEG = -30000.0

bf16 = ml_dtypes.bfloat16

_CACHE: dict = {}


def _emit(tc, nc, mybir, bass, ctx):
    dt = mybir.dt
    f32, b16 = dt.float32, dt.bfloat16
    AF = mybir.ActivationFunctionType

    xT_d = nc.dram_tensor("xT", [C, T], b16, kind="ExternalInput")
    wqk_d = nc.dram_tensor("wqk", [C, 2 * DG], b16, kind="ExternalInput")
    wv_d = nc.dram_tensor("wv", [C, DG], b16, kind="ExternalInput")
    wp_d = nc.dram_tensor("wp", [DG, C], b16, kind="ExternalInput")
    cos_d = nc.dram_tensor("cosT", [128, T], b16, kind="ExternalInput")
    sin_d = nc.dram_tensor("sinT", [128, T], b16, kind="ExternalInput")
    tri_d = nc.dram_tensor("tri", [128, 128], b16, kind="ExternalInput")
    y_d = nc.dram_tensor("y", [T, C], b16, kind="ExternalOutput")

    const = ctx.enter_context(tc.tile_pool(name="const", bufs=1))
    work = ctx.enter_context(tc.tile_pool(name="work", bufs=1))

    # ---- resident SBUF loads, ordered by first use ----
    xt_sb = const.tile([128, CT, T], b16, tag="xt")
    wqk_sb = const.tile([128, CT, 2 * DG], b16, tag="wqk")
    wv_sb = const.tile([128, CT, DG], b16, tag="wv")
    TH = T // 2
    for i in range(CT):
        # grp-0 wqk + first-half xT land first so phase A starts early
        # (host lays wqk out as [q_g0 | k_g0 | q_g1 | k_g1])
        nc.sync.dma_start(wqk_sb[:, i, 0:256],
                          wqk_d.ap()[i * 128:(i + 1) * 128, 0:256])
        nc.sync.dma_start(xt_sb[:, i, 0:TH],
                          xT_d.ap()[i * 128:(i + 1) * 128, 0:TH])
    for i in range(CT):
        nc.sync.dma_start(wqk_sb[:, i, 256:512],
                          wqk_d.ap()[i * 128:(i + 1) * 128, 256:512])
    cos_sb = const.tile([128, T], b16, tag="cos")
    nc.sync.dma_start(cos_sb[:], cos_d.ap())
    sin_sb = const.tile([128, T], b16, tag="sin")
    nc.sync.dma_start(sin_sb[:], sin_d.ap())
    for i in range(CT):
        nc.sync.dma_start(xt_sb[:, i, TH:T],
                          xT_d.ap()[i * 128:(i + 1) * 128, TH:T])
        nc.sync.dma_start(wv_sb[:, i, :], wv_d.ap()[i * 128:(i + 1) * 128, :])
    tri_sb = const.tile([128, 128], b16, tag="tri")
    nc.sync.dma_start(tri_sb[:], tri_d.ap())
    wp_sb = const.tile([128, 2, C], b16, tag="wp")
    nc.sync.dma_start(wp_sb[:], wp_d.ap().rearrange("(a p) d -> p a d", p=128))

    # rope outputs: [d, t] bf16, 2 grp-tiles each (grp = 2 heads = 128 rows)
    q_sb = work.tile([128, 2, T], b16, tag="q")
    k_sb = work.tile([128, 2, T], b16, tag="k")
    # v in [t, d] layout with per-head ones column: [t-tile, head, 65]
    v_sb = work.tile([128, NTT, HPG, HD + 1], b16, tag="v")
    # attention outputs O^T (normalized), split by query half for clean
    # tile-level deps (phase D half 0 only needs o_lo)
    o_lo = work.tile([128, 2, TH], b16, tag="olo")
    o_hi = work.tile([128, 2, TH], b16, tag="ohi")

    # ones columns only (v cols 0:64 written by phase B evacuation)
    nc.gpsimd.memset(v_sb[:, :, :, 64:65], 1.0)

    rope_pool = ctx.enter_context(tc.tile_pool(name="rope", bufs=2))

    def rope_chain(ps, d, c0, width, in_c):
        # rope: dst = raw*cos + shuf*sin' over t-columns [c0, c0+width).
        # Engine split depends on phase: pre-C the evac rides ScalarE (idle
        # then); inside C ScalarE is exp-bound so the evac goes to DVE and
        # both muls to gpsimd.
        is_q = (d % 2 == 0)   # wqk layout: [q_g0, k_g0, q_g1, k_g1]
        grp = d // 2
        hsl = slice(c0, c0 + width)
        raw = rope_pool.tile([128, TH], b16, tag="raw")
        if in_c:
            nc.vector.tensor_copy(raw[:, 0:width], ps[:])
        else:
            nc.scalar.copy(raw[:, 0:width], ps[:])
        # pair-swap partitions (d even<->odd): 32-way shuffle
        shuf = rope_pool.tile([128, TH], b16, tag="shuf")
        nc.vector.stream_shuffle(shuf[:, 0:width], raw[:, 0:width],
                                 [i ^ 1 for i in range(32)])
        t1 = rope_pool.tile([128, TH], b16, tag="t1")
        if in_c:
            nc.gpsimd.tensor_mul(t1[:, 0:width], raw[:, 0:width],
                                 cos_sb[:, hsl])
        else:
            nc.vector.tensor_mul(t1[:, 0:width], raw[:, 0:width],
                                 cos_sb[:, hsl])
        t2 = rope_pool.tile([128, TH], b16, tag="t2")
        nc.gpsimd.tensor_mul(t2[:, 0:width], shuf[:, 0:width], sin_sb[:, hsl])
        dst = (q_sb if is_q else k_sb)
        nc.vector.tensor_add(dst[:, grp, hsl], t1[:, 0:width], t2[:, 0:width])

    # ---- phase A half 0 + phase B first half (pre-attention) ----
    with (
        tc.tile_pool(name="qk_ps", bufs=3, space="PSUM") as qk_pool,
        tc.tile_pool(name="v_ps", bufs=2, space="PSUM") as v_pool,
    ):
        for half in range(2):      # [128, 1024] halves
            h0 = half * TH
            for dpair in ((0, 1), (2, 3)):  # (q, k) per grp together
                # ci-outer over a dtile pair: each arriving c-tile feeds 4
                # matmuls, so the DMA-paced kernel start keeps PE fed
                pss = {}
                for d in dpair:
                    qkps = qk_pool.tile([128, TH], f32, tag="qkps")
                    pss[d] = qkps
                for ci in range(CT):
                    for d in dpair:
                        for j in range(2):
                            nc.tensor.matmul(
                                pss[d][:, j * 512:(j + 1) * 512],
                                wqk_sb[:, ci, d * 128:(d + 1) * 128],
                                xt_sb[:, ci, h0 + j * 512:h0 + (j + 1) * 512],
                                start=(ci == 0),
                                stop=(ci == CT - 1),
                            )
                for d in dpair:
                    rope_chain(pss[d], d, h0, TH, in_c=False)

        # phase B: v in [t, d] layout (first half; rest interleaved into C)
        for tt in range(NTT // 2):
            vps = v_pool.tile([128, DG], f32, tag="vps")
            for ci in range(CT):
                nc.tensor.matmul(
                    vps[:],
                    xt_sb[:, ci, tt * 128:(tt + 1) * 128],
                    wv_sb[:, ci, :],
                    start=(ci == 0),
                    stop=(ci == CT - 1),
                )
            nc.scalar.copy(
                v_sb[:, tt, :, 0:HD],
                vps[:].rearrange("p (h d) -> p h d", h=HPG),
            )

    # ---- phase C: attention per head; phase D interleaved ----
    # PSUM budget: sps 2 bufs x 2 banks + ops 1 buf x 2 banks x... exact:
    # sps [128,1024] f32 = 2 banks (bufs=2 -> 4), ops [65,1024] f32 = 2
    # banks (bufs=2 -> 4)... that is 8; y interleave needs its own pool,
    # so ops gets bufs=1 (2 banks) and y_ps bufs=2 (2 banks).
    with (
        tc.tile_pool(name="o_ps", bufs=1, space="PSUM") as o_pool,
        tc.tile_pool(name="s_ps", bufs=2, space="PSUM") as s_pool,
        tc.tile_pool(name="y_ps", bufs=2, space="PSUM") as y_pool,
        tc.tile_pool(name="p_sb", bufs=6) as p_pool,
        tc.tile_pool(name="r_sb", bufs=2) as r_pool,
        tc.tile_pool(name="y_sb", bufs=4) as ysb_pool,
    ):
        def emit_v(tt):
            # late v tiles, interleaved into jh0 windows on borrowed y slots
            vps = y_pool.tile([128, 512], f32, tag="yps")
            for ci in range(CT):
                nc.tensor.matmul(
                    vps[:, 0:DG],
                    xt_sb[:, ci, tt * 128:(tt + 1) * 128],
                    wv_sb[:, ci, :],
                    start=(ci == 0),
                    stop=(ci == CT - 1),
                )
            nc.vector.tensor_copy(
                v_sb[:, tt, :, 0:HD],
                vps[:, 0:DG].rearrange("p (h d) -> p h d", h=HPG),
            )

        def emit_proj(tt, cc, on_dve, pool=None, spool=None):
            o_t = o_lo if tt < NTT // 2 else o_hi
            toff = tt * 128 - (0 if tt < NTT // 2 else TH)
            yps = (pool or y_pool).tile([128, 512], f32, tag="yps")
            for grp in range(2):
                nc.tensor.matmul(
                    yps[:],
                    o_t[:, grp, toff:toff + 128],
                    wp_sb[:, grp, cc * 512:(cc + 1) * 512],
                    start=(grp == 0),
                    stop=(grp == 1),
                )
            ysb = (spool or ysb_pool).tile([128, 512], b16, tag="ysb")
            if on_dve:
                nc.vector.tensor_copy(ysb[:], yps[:])
            else:
                nc.scalar.copy(ysb[:], yps[:])
            nc.sync.dma_start(
                y_d.ap()[tt * 128:(tt + 1) * 128, cc * 512:(cc + 1) * 512],
                ysb[:],
            )

        def window(jh, h):
                grp, base = h // 2, 64 * (h % 2)
                o_t = o_lo if jh == 0 else o_hi
                ops = o_pool.tile([65, 1024], f32, tag="ops")
                w0 = jh * 1024
                ilim = min(8 * jh + 8, NTT)
                for i in range(ilim):
                    woff = max(0, 128 * i - w0)  # first valid col in window
                    sps = s_pool.tile([128, 1024], f32, tag="sps")
                    klhs = k_sb[base:base + 64, grp, i * 128:(i + 1) * 128]
                    for sj in range(2):  # 512 sub-chunks (PSUM bank each)
                        j = 2 * jh + sj
                        if i > 4 * j + 3:
                            continue  # fully masked sub-chunk
                        off = max(0, 128 * i - 512 * j)
                        nc.tensor.matmul(
                            sps[:, sj * 512 + off:(sj + 1) * 512],
                            klhs,
                            q_sb[base:base + 64, grp,
                                 j * 512 + off:(j + 1) * 512],
                            start=True,
                            stop=True,
                        )
                    psb = p_pool.tile([128, 1024], b16, tag="psb")
                    nc.scalar.activation(
                        psb[:, woff:1024], sps[:, woff:1024], AF.Exp,
                        scale=0.125,
                    )
                    d0 = 128 * i - w0  # tri-block col within window
                    if 0 <= d0 <= 1024 - 128:
                        # zero the strictly-lower (q < key) part of the
                        # diagonal tile post-exp (replaces mask matmul)
                        nc.vector.tensor_mul(psb[:, d0:d0 + 128],
                                             psb[:, d0:d0 + 128], tri_sb[:])
                    for sj in range(2):
                        j = 2 * jh + sj
                        if i > 4 * j + 3:
                            continue
                        off = max(0, 128 * i - 512 * j)
                        nc.tensor.matmul(
                            ops[:, sj * 512 + off:(sj + 1) * 512],
                            v_sb[:, i, h, :],
                            psb[:, sj * 512 + off:(sj + 1) * 512],
                            start=(i == 0),
                            stop=(i == min(4 * j + 3, ilim - 1)),
                        )
                # evacuate O'^T to SBUF in one DVE copy so the ops PSUM
                # slot frees fast (o_pool bufs=1), then normalize from SBUF
                oev = r_pool.tile([65, 1024], dt.float32, tag="oev")
                nc.vector.tensor_copy(oev[:], ops[:])
                rec = r_pool.tile([1, 1024], dt.float32, tag="rec")
                nc.vector.reciprocal(rec[:], oev[64:65, :])
                rrep = r_pool.tile([64, 1024], dt.float32, tag="rrep")
                nc.gpsimd.partition_broadcast(rrep[:], rec[:])
                nc.vector.tensor_mul(o_t[base:base + 64, grp, 0:1024],
                                     oev[0:64, :], rrep[:])
                if jh == 0:
                    # late v tiles on PE while ACT chews exps
                    emit_v(8 + 2 * h)
                    emit_v(9 + 2 * h)
                else:
                    # interleave phase-D half 0 (reads o_lo only) into the
                    # ACT-bound attention stretch
                    for tt in (2 * h, 2 * h + 1):
                        emit_proj(tt, 0, True)
                        emit_proj(tt, 1, True)

        for h in range(HPG):
            window(0, h)

        for h in range(HPG):
            window(1, h)

    # ---- phase D tail: second query half ----
    # separate pool block: reuses the freed attention PSUM banks for a
    # deeper projection pipeline
    with (
        tc.tile_pool(name="y2_ps", bufs=6, space="PSUM") as y2_pool,
        tc.tile_pool(name="y2_sb", bufs=8) as ysb2_pool,
    ):
        for tt in range(NTT // 2, NTT):
            for cc in range(2):
                emit_proj(tt, cc, cc == 1, pool=y2_pool, spool=ysb2_pool)


def build_program():
    if "nc" in _CACHE:
        return _CACHE["nc"]
    import concourse.bass as bass
    import concourse.bacc as bacc
    import concourse.tile as tile
    import concourse.mybir as mybir

    nc = bacc.Bacc("TRN2", target_bir_lowering=False, debug=False,
                   enable_asserts=True)
    with tile.TileContext(nc) as tc:
        with ExitStack() as ctx:
            _emit(tc, nc, mybir, bass, ctx)
    nc.compile()
    _CACHE["nc"] = nc
    return nc


def make_tables():
    """cos/sin tables ([128, T], two 64-row head copies) and tri mask."""
    if "tables" in _CACHE:
        return _CACHE["tables"]
    hd = HD
    inv_freq = 1.0 / (10000.0 ** (np.arange(0, hd, 2, dtype=np.float64) / hd))
    t = np.arange(T, dtype=np.float64)
    emb = t[:, None] * np.concatenate([inv_freq, inv_freq])[None, :]  # [T, 64]
    cos = np.cos(emb).T.astype(np.float32)       # [64, T]
    sin = np.sin(emb).T.astype(np.float32)
    sign = np.where(np.arange(hd) % 2 == 0, -1.0, 1.0).astype(np.float32)
    sin = sin * sign[:, None]
    cos128 = np.concatenate([cos, cos], axis=0).astype(bf16)   # [128, T]
    sin128 = np.concatenate([sin, sin], axis=0).astype(bf16)
    ii = np.arange(128)
    # tri[k, q] = 1 where q >= k (valid causal), else 0
    tri = (ii[None, :] >= ii[:, None]).astype(bf16)
    _CACHE["tables"] = (cos128, sin128, tri)
    return _CACHE["tables"]


def make_in_maps(x, w_qkv, w_proj):
    cos128, sin128, tri = make_tables()
    wq = w_qkv[:, 0:C]
    wk = w_qkv[:, C:2 * C]
    wv = w_qkv[:, 2 * C:3 * C]
    in_maps = []
    for b in range(B):
        xT = np.ascontiguousarray(x[b].T).astype(bf16)
        for g in range(GROUPS):
            sl = slice(g * DG, (g + 1) * DG)
            sg = [slice(g * DG + p * HD * 2, g * DG + (p + 1) * HD * 2)
                  for p in range(2)]
            in_maps.append({
                "xT": xT,
                "wqk": np.concatenate(
                    [wq[:, sg[0]], wk[:, sg[0]], wq[:, sg[1]], wk[:, sg[1]]],
                    axis=1).astype(bf16),
                "wv": wv[:, sl].astype(bf16),
                "wp": w_proj[sl, :].astype(bf16),
                "cosT": cos128, "sinT": sin128, "tri": tri,
            })
    return in_maps


def kernel(x, w_qkv, w_proj):
    from concourse import bass_utils
    nc = build_program()
    in_maps = make_in_maps(np.asarray(x, dtype=np.float32),
                           np.asarray(w_qkv, dtype=np.float32),
                           np.asarray(w_proj, dtype=np.float32))
    res = bass_utils.run_bass_kernel_spmd(nc, in_maps, list(range(NCORES)))
    out = np.empty((B, T, C), dtype=np.float32)
    for b in range(B):
        acc = np.zeros((T, C), dtype=np.float32)
        for g in range(GROUPS):
            acc += np.asarray(res.results[b * GROUPS + g]["y"], dtype=np.float32)
        out[b] = acc
    return out
